# revision 10
# baseline (speedup 1.0000x reference)
# Trainium2 Bass kernel for the EmbodiedCTRNN problem.
#
# Model (reference semantics):
#   x_proj = einsum("tbi,hi->tbh", x, W_i2h) + b_i2h
#   step t: pre   = x_proj[t] + h @ W_h2h.T + b_h2h + b @ W_b2h.T + b_b2h
#           h_new = h*(1-a) + relu(pre)*a
#           b_new = b*mask + a*(h @ W_h2b.T + b_h2b)
#   outputs: hidden_out [T,B,H], body_out [T,B,BODY], h_fin [B,H]
#
# Strategy: data-parallel over batch (B=256 -> 8 cores x 32). Per core the
# state is kept transposed in SBUF ([H-on-partitions, batch-in-free]) so the
# per-step vector/scalar ops are 128-partition wide and cheap. The input
# projection GEMM is computed on-device in T-chunks (fp32r, N=512 moving) and
# consumed from SBUF; the recurrence accumulates h2h/b2h into PSUM per
# 128-row output chunk. Outputs are written transposed ([H, T, b]) with
# contiguous DMA runs and re-transposed on the host during unsharding.

import numpy as np

T, B, IN, H, BODY = 512, 256, 128, 512, 64
ALPHA = 0.1
NCORES = 8
BL = B // NCORES          # 32 batch per core
NCH = 4                   # H / 128 chunks
CH = 32                   # timesteps per x_proj GEMM chunk (CH*BL = 1024 free)
S = 16                    # timesteps per output-staging block
GEMM_N = 512              # moving free-dim per GEMM matmul

F32 = None  # set lazily (mybir import inside functions)


def _build(t_total=T, recur_bf16=False, gemm_f32r=True):
    import concourse.bass as bass
    import concourse.mybir as mybir
    from concourse import bacc
    from concourse.bass import ts
    from concourse.tile import TileContext

    f32 = mybir.dt.float32
    f32r = mybir.dt.float32r
    bf16 = mybir.dt.bfloat16
    wdt = bf16 if recur_bf16 else f32
    AF = mybir.ActivationFunctionType
    ALU = mybir.AluOpType

    nc = bacc.Bacc(None, target_bir_lowering=False)

    xdt = f32r if gemm_f32r else f32
    xT_d = nc.dram_tensor("xT", [IN, t_total * BL], xdt, kind="ExternalInput")
    whh_d = nc.dram_tensor("whhT", [NCH, 128, H], wdt, kind="ExternalInput")
    wi_d = nc.dram_tensor("wi2hT", [IN, H], xdt, kind="ExternalInput")
    wb_d = nc.dram_tensor("wb2hT", [BODY, H], wdt, kind="ExternalInput")
    whb_d = nc.dram_tensor("wh2bT", [NCH, 128, BODY], wdt, kind="ExternalInput")
    hbias_d = nc.dram_tensor("hbias", [128, NCH], f32, kind="ExternalInput")
    bbias_d = nc.dram_tensor("bbias", [BODY, 1], f32, kind="ExternalInput")
    mask_d = nc.dram_tensor("maskT", [BODY, 1], f32, kind="ExternalInput")

    hoT_d = nc.dram_tensor("hoT", [NCH, 128, t_total, BL], f32, kind="ExternalOutput")
    boT_d = nc.dram_tensor("boT", [BODY, t_total, BL], f32, kind="ExternalOutput")

    assert t_total % CH == 0 and CH % S == 0
    n_chunks = t_total // CH
    blk_steps = GEMM_N // BL  # timesteps covered by one GEMM matmul (16)

    with TileContext(nc) as tc:
        with (
            tc.tile_pool(name="weights", bufs=1) as wpool,
            tc.tile_pool(name="xt", bufs=2) as xt_pool,
            tc.tile_pool(name="xp", bufs=2) as xp_pool,
            tc.tile_pool(name="hst", bufs=2) as hst_pool,
            tc.tile_pool(name="bst", bufs=2) as bst_pool,
            tc.tile_pool(name="rt", bufs=2) as rt_pool,
            tc.tile_pool(name="rbt", bufs=2) as rbt_pool,
            tc.tile_pool(name="psum_pre", bufs=4, space="PSUM") as pre_pool,
            tc.tile_pool(name="psum_b", bufs=2, space="PSUM") as pb_pool,
            tc.tile_pool(name="psum_g", bufs=2, space="PSUM") as pg_pool,
        ):
            # --- resident weights ---
            whh_sb = wpool.tile([128, NCH, H], wdt, tag="whh")
            for j in range(NCH):
                nc.sync.dma_start(out=whh_sb[:, j, :], in_=whh_d[j])
            wi_sb = wpool.tile([IN, H], xdt, tag="wi")
            nc.sync.dma_start(out=wi_sb[:], in_=wi_d[:])
            wb_sb = wpool.tile([BODY, H], wdt, tag="wb")
            nc.sync.dma_start(out=wb_sb[:], in_=wb_d[:])
            whb_sb = wpool.tile([128, NCH, BODY], wdt, tag="whb")
            for j in range(NCH):
                nc.sync.dma_start(out=whb_sb[:, j, :], in_=whb_d[j])
            hbias_sb = wpool.tile([128, NCH], f32, tag="hbias")
            nc.sync.dma_start(out=hbias_sb[:], in_=hbias_d[:])
            bbias_sb = wpool.tile([BODY, 1], f32, tag="bbias")
            nc.sync.dma_start(out=bbias_sb[:], in_=bbias_d[:])
            mask_sb = wpool.tile([BODY, 1], f32, tag="mask")
            nc.sync.dma_start(out=mask_sb[:], in_=mask_d[:])

            prev_h = None  # AP of last written h state slot [128, NCH, BL]
            prev_b = None

            for c in range(n_chunks):
                t0 = c * CH
                # --- load x^T chunk and compute x_proj^T chunk into SBUF ---
                xt = xt_pool.tile([IN, CH * BL], xdt, tag="xt")
                nc.sync.dma_start(out=xt[:], in_=xT_d[:, t0 * BL : (t0 + CH) * BL])
                xp = xp_pool.tile([128, NCH, CH * BL], f32, tag="xp")
                for blk in range(CH * BL // GEMM_N):
                    for m in range(NCH):
                        pg = pg_pool.tile([128, GEMM_N], f32, tag="pg")
                        nc.tensor.matmul(
                            pg[:], wi_sb[:, ts(m, 128)], xt[:, ts(blk, GEMM_N)]
                        )
                        nc.vector.tensor_copy(xp[:, m, ts(blk, GEMM_N)], pg[:])

                # --- recurrence over this chunk, in S-step staging blocks ---
                for sb in range(CH // S):
                    hst = hst_pool.tile([128, NCH, S + 1, BL], f32, tag="hst")
                    bst = bst_pool.tile([BODY, S + 1, BL], f32, tag="bst")
                    if recur_bf16:
                        hstB = hst_pool.tile([128, NCH, S + 1, BL], bf16, tag="hstB")
                        bstB = bst_pool.tile([BODY, S + 1, BL], bf16, tag="bstB")
                    if prev_h is None:
                        nc.vector.memset(hst[:, :, 0, :], 0.0)
                        nc.vector.memset(bst[:, 0, :], 0.0)
                        if recur_bf16:
                            nc.vector.memset(hstB[:, :, 0, :], 0.0)
                            nc.vector.memset(bstB[:, 0, :], 0.0)
                    else:
                        nc.vector.tensor_copy(hst[:, :, 0, :], prev_h)
                        nc.vector.tensor_copy(bst[:, 0, :], prev_b)
                        if recur_bf16:
                            nc.vector.tensor_copy(hstB[:, :, 0, :], prev_h)
                            nc.vector.tensor_copy(bstB[:, 0, :], prev_b)

                    mv_h = hstB if recur_bf16 else hst
                    mv_b = bstB if recur_bf16 else bst

                    for s in range(S):
                        tl = sb * S + s  # step within chunk
                        # body-state matmul group: psum_b = W_h2b @ h
                        pb = pb_pool.tile([BODY, BL], f32, tag="pb")
                        for j in range(NCH):
                            nc.tensor.matmul(
                                pb[:],
                                whb_sb[:, j, :],
                                mv_h[:, j, s, :],
                                start=(j == 0),
                                stop=(j == NCH - 1),
                            )
                        # hidden pre-activation chunks
                        pres = []
                        for m in range(NCH):
                            pp = pre_pool.tile([128, BL], f32, tag="pre")
                            pres.append(pp)
                            for j in range(NCH):
                                nc.tensor.matmul(
                                    pp[:],
                                    whh_sb[:, j, ts(m, 128)],
                                    mv_h[:, j, s, :],
                                    start=(j == 0),
                                    stop=False,
                                )
                            nc.tensor.matmul(
                                pp[:],
                                wb_sb[:, ts(m, 128)],
                                mv_b[:, s, :],
                                start=False,
                                stop=True,
                            )
                        rt = rt_pool.tile([128, NCH, BL], f32, tag="rt")
                        for m in range(NCH):
                            pp = pres[m]
                            # pre += x_proj (psum in-place)
                            nc.vector.tensor_add(
                                pp[:], pp[:], xp[:, m, (tl * BL) : (tl + 1) * BL]
                            )
                            # r = relu(alpha*pre + alpha*bias)
                            nc.scalar.activation(
                                rt[:, m, :],
                                pp[:],
                                AF.Relu,
                                bias=hbias_sb[:, m : m + 1],
                                scale=ALPHA,
                            )
                            # h_new = 0.9*h + r
                            nc.vector.scalar_tensor_tensor(
                                hst[:, m, s + 1, :],
                                hst[:, m, s, :],
                                1.0 - ALPHA,
                                rt[:, m, :],
                                op0=ALU.mult,
                                op1=ALU.add,
                            )
                        # body update: b_new = b*mask + alpha*(psum_b + b_h2b)
                        rbt = rbt_pool.tile([BODY, BL], f32, tag="rbt")
                        nc.vector.tensor_scalar(
                            rbt[:], pb[:], bbias_sb[:, 0:1], ALPHA,
                            op0=ALU.add, op1=ALU.mult,
                        )
                        nc.vector.scalar_tensor_tensor(
                            bst[:, s + 1, :],
                            bst[:, s, :],
                            mask_sb[:, 0:1],
                            rbt[:],
                            op0=ALU.mult,
                            op1=ALU.add,
                        )
                        if recur_bf16:
                            nc.vector.tensor_copy(
                                hstB[:, :, s + 1, :], hst[:, :, s + 1, :]
                            )
                            nc.vector.tensor_copy(bstB[:, s + 1, :], bst[:, s + 1, :])

                    # --- stage out this block ---
                    tg = t0 + sb * S
                    for m in range(NCH):
                        nc.sync.dma_start(
                            out=hoT_d[m, :, tg : tg + S, :],
                            in_=hst[:, m, 1 : S + 1, :],
                        )
                    nc.sync.dma_start(
                        out=boT_d[:, tg : tg + S, :], in_=bst[:, 1 : S + 1, :]
                    )
                    prev_h = hst[:, :, S, :]
                    prev_b = bst[:, S, :]

    nc.finalize()
    return nc


def _prep_inputs(inputs, t_total=T, recur_bf16=False):
    import ml_dtypes

    wdt = ml_dtypes.bfloat16 if recur_bf16 else np.float32
    x = np.asarray(inputs["x"], np.float32)[:t_total]
    W_i2h = np.asarray(inputs["W_i2h"], np.float32)
    b_i2h = np.asarray(inputs["b_i2h"], np.float32)
    W_h2h = np.asarray(inputs["W_h2h"], np.float32)
    b_h2h = np.asarray(inputs["b_h2h"], np.float32)
    W_b2h = np.asarray(inputs["W_b2h"], np.float32)
    b_b2h = np.asarray(inputs["b_b2h"], np.float32)
    W_h2b = np.asarray(inputs["W_h2b"], np.float32)
    b_h2b = np.asarray(inputs["b_h2b"], np.float32)
    mask = np.asarray(inputs["body_mask"], np.float32)

    shared = {
        "whhT": np.ascontiguousarray(
            W_h2h.T.reshape(NCH, 128, H).astype(wdt)
        ),
        "wi2hT": np.ascontiguousarray(W_i2h.T),
        "wb2hT": np.ascontiguousarray(W_b2h.T.astype(wdt)),
        "wh2bT": np.ascontiguousarray(W_h2b.T.reshape(NCH, 128, BODY).astype(wdt)),
        "hbias": np.ascontiguousarray(
            (ALPHA * (b_i2h + b_h2h + b_b2h)).reshape(NCH, 128).T
        ),
        "bbias": np.ascontiguousarray(b_h2b.reshape(BODY, 1)),
        "maskT": np.ascontiguousarray(mask.reshape(BODY, 1)),
    }
    in_maps = []
    for c in range(NCORES):
        xc = x[:, c * BL : (c + 1) * BL, :]  # [T, BL, IN]
        xT = np.ascontiguousarray(xc.transpose(2, 0, 1).reshape(IN, t_total * BL))
        in_maps.append({"xT": xT, **shared})
    return in_maps


def _assemble(results, t_total=T):
    hidden = np.empty((t_total, B, H), np.float32)
    body = np.empty((t_total, B, BODY), np.float32)
    for c, res in enumerate(results):
        hoT = res["hoT"]  # [NCH, 128, T, BL]
        boT = res["boT"]  # [BODY, T, BL]
        hidden[:, c * BL : (c + 1) * BL, :] = (
            hoT.transpose(2, 3, 0, 1).reshape(t_total, BL, H)
        )
        body[:, c * BL : (c + 1) * BL, :] = boT.transpose(1, 2, 0)
    h_fin = hidden[-1].copy()
    return hidden, body, h_fin


def kernel(**inputs):
    from concourse.bass_utils import run_bass_kernel_spmd

    recur_bf16 = False
    nc = _build(T, recur_bf16=recur_bf16)
    in_maps = _prep_inputs(inputs, T, recur_bf16=recur_bf16)
    out = run_bass_kernel_spmd(nc, in_maps, core_ids=list(range(NCORES)))
    return _assemble(out.results, T)


# revision 11
# speedup vs baseline: 3.7619x; 3.7619x over previous
# Trainium2 Bass kernel for the EmbodiedCTRNN problem.
#
# Model (reference semantics):
#   x_proj = einsum("tbi,hi->tbh", x, W_i2h) + b_i2h
#   step t: pre   = x_proj[t] + h @ W_h2h.T + b_h2h + b @ W_b2h.T + b_b2h
#           h_new = h*(1-a) + relu(pre)*a
#           b_new = b*mask + a*(h @ W_h2b.T + b_h2b)
#   outputs: hidden_out [T,B,H], body_out [T,B,BODY], h_fin [B,H]
#
# Strategy: data-parallel over batch (B=256 -> 8 cores x 32). Per core the
# state is kept transposed in SBUF ([H-on-partitions, batch-in-free]) so the
# per-step vector/scalar ops are 128-partition wide and cheap. The input
# projection GEMM is computed on-device in T-chunks (fp32r, N=512 moving) and
# consumed from SBUF; the recurrence accumulates h2h/b2h into PSUM per
# 128-row output chunk. Outputs are written transposed ([H, T, b]) with
# contiguous DMA runs and re-transposed on the host during unsharding.

import numpy as np

T, B, IN, H, BODY = 512, 256, 128, 512, 64
ALPHA = 0.1
NCORES = 8
BL = B // NCORES          # 32 batch per core
NCH = 4                   # H / 128 chunks
CH = 32                   # timesteps per x_proj GEMM chunk (CH*BL = 1024 free)
S = 16                    # timesteps per output-staging block
GEMM_N = 512              # moving free-dim per GEMM matmul

F32 = None  # set lazily (mybir import inside functions)


def _build(t_total=T, recur_bf16=False, gemm_f32r=True):
    import concourse.bass as bass
    import concourse.mybir as mybir
    from concourse import bacc
    from concourse.bass import ts
    from concourse.tile import TileContext

    f32 = mybir.dt.float32
    f32r = mybir.dt.float32r
    bf16 = mybir.dt.bfloat16
    wdt = bf16 if recur_bf16 else f32
    AF = mybir.ActivationFunctionType
    ALU = mybir.AluOpType

    nc = bacc.Bacc(None, target_bir_lowering=False)

    xdt = f32r if gemm_f32r else f32
    xT_d = nc.dram_tensor("xT", [IN, t_total * BL], xdt, kind="ExternalInput")
    whh_d = nc.dram_tensor("whhT", [NCH, 128, H], wdt, kind="ExternalInput")
    wi_d = nc.dram_tensor("wi2hT", [IN, H], xdt, kind="ExternalInput")
    wb_d = nc.dram_tensor("wb2hT", [BODY, H], wdt, kind="ExternalInput")
    whb_d = nc.dram_tensor("wh2bT", [NCH, 128, BODY], wdt, kind="ExternalInput")
    hbias_d = nc.dram_tensor("hbias", [128, NCH], f32, kind="ExternalInput")
    bbias_d = nc.dram_tensor("bbias", [BODY, 1], f32, kind="ExternalInput")
    mask_d = nc.dram_tensor("maskT", [BODY, 1], f32, kind="ExternalInput")

    hoT_d = nc.dram_tensor("hoT", [NCH, 128, t_total, BL], f32, kind="ExternalOutput")
    boT_d = nc.dram_tensor("boT", [BODY, t_total, BL], f32, kind="ExternalOutput")

    assert t_total % CH == 0 and CH % S == 0
    n_chunks = t_total // CH
    blk_steps = GEMM_N // BL  # timesteps covered by one GEMM matmul (16)

    with TileContext(nc) as tc:
        with (
            tc.tile_pool(name="weights", bufs=1) as wpool,
            tc.tile_pool(name="xt", bufs=2) as xt_pool,
            tc.tile_pool(name="xp", bufs=2) as xp_pool,
            tc.tile_pool(name="hst", bufs=2) as hst_pool,
            tc.tile_pool(name="bst", bufs=2) as bst_pool,
            tc.tile_pool(name="rt", bufs=2) as rt_pool,
            tc.tile_pool(name="rbt", bufs=2) as rbt_pool,
            tc.tile_pool(name="psum_pre", bufs=4, space="PSUM") as pre_pool,
            tc.tile_pool(name="psum_b", bufs=2, space="PSUM") as pb_pool,
            tc.tile_pool(name="psum_g", bufs=2, space="PSUM") as pg_pool,
        ):
            # --- resident weights ---
            whh_sb = wpool.tile([128, NCH, H], wdt, tag="whh")
            for j in range(NCH):
                nc.sync.dma_start(out=whh_sb[:, j, :], in_=whh_d[j])
            wi_sb = wpool.tile([IN, H], xdt, tag="wi")
            nc.sync.dma_start(out=wi_sb[:], in_=wi_d[:])
            wb_sb = wpool.tile([BODY, H], wdt, tag="wb")
            nc.sync.dma_start(out=wb_sb[:], in_=wb_d[:])
            whb_sb = wpool.tile([128, NCH, BODY], wdt, tag="whb")
            for j in range(NCH):
                nc.sync.dma_start(out=whb_sb[:, j, :], in_=whb_d[j])
            hbias_sb = wpool.tile([128, NCH], f32, tag="hbias")
            nc.sync.dma_start(out=hbias_sb[:], in_=hbias_d[:])
            bbias_sb = wpool.tile([BODY, 1], f32, tag="bbias")
            nc.sync.dma_start(out=bbias_sb[:], in_=bbias_d[:])
            mask_sb = wpool.tile([BODY, 1], f32, tag="mask")
            nc.sync.dma_start(out=mask_sb[:], in_=mask_d[:])

            prev_h = None  # AP of last written h state slot [128, NCH, BL]
            prev_b = None

            for c in range(n_chunks):
                t0 = c * CH
                # --- load x^T chunk and compute x_proj^T chunk into SBUF ---
                xt = xt_pool.tile([IN, CH * BL], xdt, tag="xt")
                nc.sync.dma_start(out=xt[:], in_=xT_d[:, t0 * BL : (t0 + CH) * BL])
                xp = xp_pool.tile([128, NCH, CH * BL], f32, tag="xp")
                for blk in range(CH * BL // GEMM_N):
                    for m in range(NCH):
                        pg = pg_pool.tile([128, GEMM_N], f32, tag="pg")
                        nc.tensor.matmul(
                            pg[:], wi_sb[:, ts(m, 128)], xt[:, ts(blk, GEMM_N)]
                        )
                        nc.vector.tensor_copy(xp[:, m, ts(blk, GEMM_N)], pg[:])

                # --- recurrence over this chunk, in S-step staging blocks ---
                for sb in range(CH // S):
                    hst = hst_pool.tile([128, NCH, S + 1, BL], f32, tag="hst")
                    bst = bst_pool.tile([BODY, S + 1, BL], f32, tag="bst")
                    if recur_bf16:
                        hstB = hst_pool.tile([128, NCH, S + 1, BL], bf16, tag="hstB")
                        bstB = bst_pool.tile([BODY, S + 1, BL], bf16, tag="bstB")
                    if prev_h is None:
                        nc.vector.memset(hst[:, :, 0, :], 0.0)
                        nc.vector.memset(bst[:, 0, :], 0.0)
                        if recur_bf16:
                            nc.vector.memset(hstB[:, :, 0, :], 0.0)
                            nc.vector.memset(bstB[:, 0, :], 0.0)
                    else:
                        nc.vector.tensor_copy(hst[:, :, 0, :], prev_h)
                        nc.vector.tensor_copy(bst[:, 0, :], prev_b)
                        if recur_bf16:
                            nc.vector.tensor_copy(hstB[:, :, 0, :], prev_h)
                            nc.vector.tensor_copy(bstB[:, 0, :], prev_b)

                    mv_h = hstB if recur_bf16 else hst
                    mv_b = bstB if recur_bf16 else bst

                    for s in range(S):
                        tl = sb * S + s  # step within chunk
                        # body-state matmul group: psum_b = W_h2b @ h
                        pb = pb_pool.tile([BODY, BL], f32, tag="pb")
                        for j in range(NCH):
                            nc.tensor.matmul(
                                pb[:],
                                whb_sb[:, j, :],
                                mv_h[:, j, s, :],
                                start=(j == 0),
                                stop=(j == NCH - 1),
                            )
                        # hidden pre-activation chunks
                        pres = []
                        for m in range(NCH):
                            pp = pre_pool.tile([128, BL], f32, tag="pre")
                            pres.append(pp)
                            for j in range(NCH):
                                nc.tensor.matmul(
                                    pp[:],
                                    whh_sb[:, j, ts(m, 128)],
                                    mv_h[:, j, s, :],
                                    start=(j == 0),
                                    stop=False,
                                )
                            nc.tensor.matmul(
                                pp[:],
                                wb_sb[:, ts(m, 128)],
                                mv_b[:, s, :],
                                start=False,
                                stop=True,
                            )
                        rt = rt_pool.tile([128, NCH, BL], f32, tag="rt")
                        for m in range(NCH):
                            pp = pres[m]
                            # pre += x_proj (psum in-place)
                            nc.vector.tensor_add(
                                pp[:], pp[:], xp[:, m, (tl * BL) : (tl + 1) * BL]
                            )
                            # r = relu(alpha*pre + alpha*bias)
                            nc.scalar.activation(
                                rt[:, m, :],
                                pp[:],
                                AF.Relu,
                                bias=hbias_sb[:, m : m + 1],
                                scale=ALPHA,
                            )
                            # h_new = 0.9*h + r
                            nc.vector.scalar_tensor_tensor(
                                hst[:, m, s + 1, :],
                                hst[:, m, s, :],
                                1.0 - ALPHA,
                                rt[:, m, :],
                                op0=ALU.mult,
                                op1=ALU.add,
                            )
                        # body update: b_new = b*mask + alpha*(psum_b + b_h2b)
                        rbt = rbt_pool.tile([BODY, BL], f32, tag="rbt")
                        nc.vector.tensor_scalar(
                            rbt[:], pb[:], bbias_sb[:, 0:1], ALPHA,
                            op0=ALU.add, op1=ALU.mult,
                        )
                        nc.vector.scalar_tensor_tensor(
                            bst[:, s + 1, :],
                            bst[:, s, :],
                            mask_sb[:, 0:1],
                            rbt[:],
                            op0=ALU.mult,
                            op1=ALU.add,
                        )
                        if recur_bf16:
                            nc.vector.tensor_copy(
                                hstB[:, :, s + 1, :], hst[:, :, s + 1, :]
                            )
                            nc.vector.tensor_copy(bstB[:, s + 1, :], bst[:, s + 1, :])

                    # --- stage out this block ---
                    tg = t0 + sb * S
                    for m in range(NCH):
                        nc.sync.dma_start(
                            out=hoT_d[m, :, tg : tg + S, :],
                            in_=hst[:, m, 1 : S + 1, :],
                        )
                    nc.sync.dma_start(
                        out=boT_d[:, tg : tg + S, :], in_=bst[:, 1 : S + 1, :]
                    )
                    prev_h = hst[:, :, S, :]
                    prev_b = bst[:, S, :]

    nc.finalize()
    return nc


def _prep_inputs(inputs, t_total=T, recur_bf16=False):
    import ml_dtypes

    wdt = ml_dtypes.bfloat16 if recur_bf16 else np.float32
    x = np.asarray(inputs["x"], np.float32)[:t_total]
    W_i2h = np.asarray(inputs["W_i2h"], np.float32)
    b_i2h = np.asarray(inputs["b_i2h"], np.float32)
    W_h2h = np.asarray(inputs["W_h2h"], np.float32)
    b_h2h = np.asarray(inputs["b_h2h"], np.float32)
    W_b2h = np.asarray(inputs["W_b2h"], np.float32)
    b_b2h = np.asarray(inputs["b_b2h"], np.float32)
    W_h2b = np.asarray(inputs["W_h2b"], np.float32)
    b_h2b = np.asarray(inputs["b_h2b"], np.float32)
    mask = np.asarray(inputs["body_mask"], np.float32)

    shared = {
        "whhT": np.ascontiguousarray(
            W_h2h.T.reshape(NCH, 128, H).astype(wdt)
        ),
        "wi2hT": np.ascontiguousarray(W_i2h.T),
        "wb2hT": np.ascontiguousarray(W_b2h.T.astype(wdt)),
        "wh2bT": np.ascontiguousarray(W_h2b.T.reshape(NCH, 128, BODY).astype(wdt)),
        "hbias": np.ascontiguousarray(
            (ALPHA * (b_i2h + b_h2h + b_b2h)).reshape(NCH, 128).T
        ),
        "bbias": np.ascontiguousarray(b_h2b.reshape(BODY, 1)),
        "maskT": np.ascontiguousarray(mask.reshape(BODY, 1)),
    }
    in_maps = []
    for c in range(NCORES):
        xc = x[:, c * BL : (c + 1) * BL, :]  # [T, BL, IN]
        xT = np.ascontiguousarray(xc.transpose(2, 0, 1).reshape(IN, t_total * BL))
        in_maps.append({"xT": xT, **shared})
    return in_maps


def _assemble(results, t_total=T):
    hidden = np.empty((t_total, B, H), np.float32)
    body = np.empty((t_total, B, BODY), np.float32)
    for c, res in enumerate(results):
        hoT = res["hoT"]  # [NCH, 128, T, BL]
        boT = res["boT"]  # [BODY, T, BL]
        hidden[:, c * BL : (c + 1) * BL, :] = (
            hoT.transpose(2, 3, 0, 1).reshape(t_total, BL, H)
        )
        body[:, c * BL : (c + 1) * BL, :] = boT.transpose(1, 2, 0)
    h_fin = hidden[-1].copy()
    return hidden, body, h_fin


RECUR_BF16 = True


def kernel(**inputs):
    from concourse.bass_utils import run_bass_kernel_spmd

    recur_bf16 = RECUR_BF16
    nc = _build(T, recur_bf16=recur_bf16)
    in_maps = _prep_inputs(inputs, T, recur_bf16=recur_bf16)
    out = run_bass_kernel_spmd(nc, in_maps, core_ids=list(range(NCORES)))
    return _assemble(out.results, T)
